# revision 11
# baseline (speedup 1.0000x reference)
"""2D valid cross-correlation (16x16 kernel, single channel) on 8 TRN2 cores.

Strategy: shard output rows across the 8 cores; each core's input slice
carries a 15-row halo (built host-side from the full image, so no on-device
halo exchange is needed). On each core the conv is computed on the tensor
engine as banded-Toeplitz matmuls contracting over image rows:

    y[i0+m, j0+n] = sum_b sum_k T_b[k, m] * x[i0+k, j0+n+b]

where T_b[k, m] = w[k-m, b] for 0 <= k-m < 16 (built host-side from the
runtime weights). For each output tile of [113 rows x 512 cols], 16
matmuls (one per kernel column b) accumulate into one PSUM bank; the shift
by b is a free-dim offset on the rhs access pattern, so the same SBUF
image tile serves all 16 taps. float32r runs the PE at 1 cycle/row
(~3.2x faster than fp32 matmul) at ~1e-3 scale-relative accuracy.

Output rows of a block are evicted into one wide SBUF tile and stored as a
single contiguous DRAM transfer per block (strided small stores serialize
onto one DMA engine at ~22 GB/s; contiguous transfers fan out across SDMA
engines). The scratch [block, 113*4096] layout is re-stitched on the host.
"""

from contextlib import ExitStack

import numpy as np

import concourse.tile as tile
from concourse import bacc, mybir
from concourse import bass_utils

H = W = 4096
KH = KW = 16
OH = OW = H - KH + 1          # 4081
NCORES = 8
RPC = 511                     # output rows per core (8*511 = 4088 >= 4081)
M_BLK = 113                   # output rows per lhsT block (128 - (KH-1))
T_STRIDE = 114                # per-tap stride in the Toeplitz tile (8B-aligned)
IN_ROWS = RPC + KH - 1        # 526 input rows per core (15-row halo)
N_TILE = 512                  # output cols per PSUM tile (one fp32 bank)
N_BLOCKS = (RPC + M_BLK - 1) // M_BLK  # 5 (last block: m=59)
# fp32r matmuls need even free counts; last tile is 498 wide with a
# 1-column overlap (col 3583 written twice with the identical value).
W_TILES = [(j, 512) for j in range(0, 3584, 512)] + [(3583, 498)]

F32 = mybir.dt.float32
F32R = mybir.dt.float32r

_cache = {}


def _build(mm_dt):
    nc = bacc.Bacc("TRN2", target_bir_lowering=False, debug=False)
    x_d = nc.dram_tensor("x", [IN_ROWS, W], mm_dt, kind="ExternalInput")
    t_d = nc.dram_tensor("tw", [128, KW * T_STRIDE], mm_dt, kind="ExternalInput")
    b_d = nc.dram_tensor("bias", [128, 1], F32, kind="ExternalInput")
    y_d = nc.dram_tensor("y", [N_BLOCKS, M_BLK * W], F32, kind="ExternalOutput")

    with tile.TileContext(nc) as tc, ExitStack() as ctx:
        const_pool = ctx.enter_context(tc.tile_pool(name="const", bufs=1))
        x_pool = ctx.enter_context(tc.tile_pool(name="xblk", bufs=2))
        ev_pool = ctx.enter_context(tc.tile_pool(name="evict", bufs=3))
        ps_pool = ctx.enter_context(tc.tile_pool(name="acc", bufs=4, space="PSUM"))

        # block 0's image rows first: they gate the first matmul
        x_tiles = []
        for t in range(N_BLOCKS):
            m = min(M_BLK, RPC - t * M_BLK)
            k = m + KH - 1
            if t < 2:
                x_t = x_pool.tile([128, W], mm_dt, tag="xblk")
                nc.sync.dma_start(x_t[:k, :], x_d[t * M_BLK : t * M_BLK + k, :])
                x_tiles.append(x_t)

        t_t = const_pool.tile([128, KW * T_STRIDE], mm_dt)
        nc.sync.dma_start(t_t[:], t_d[:])
        b_t = const_pool.tile([128, 1], F32)
        nc.sync.dma_start(b_t[:], b_d[:])

        for t in range(N_BLOCKS):
            m = min(M_BLK, RPC - t * M_BLK)
            k = m + KH - 1
            if t < 2:
                x_t = x_tiles[t]
            else:
                x_t = x_pool.tile([128, W], mm_dt, tag="xblk")
                nc.sync.dma_start(x_t[:k, :], x_d[t * M_BLK : t * M_BLK + k, :])
            o_t = ev_pool.tile([M_BLK, W], F32, tag="out")
            # process width tiles in pairs sharing each weight load: denser
            # PE work between PSUM switches, LDWEIGHTS amortized over 2 MMs
            for p in range(0, len(W_TILES), 2):
                pair = W_TILES[p : p + 2]
                accs = [
                    ps_pool.tile([M_BLK, N_TILE], F32, tag="acc", name=f"acc_{t}_{p}_{i}")
                    for i in range(len(pair))
                ]
                for b in range(KW):
                    for (j0, nj), acc in zip(pair, accs):
                        nc.tensor.matmul(
                            acc[:m, :nj],
                            t_t[:k, b * T_STRIDE : b * T_STRIDE + m],
                            x_t[:k, j0 + b : j0 + b + nj],
                            start=(b == 0),
                            stop=(b == KW - 1),
                        )
                for (j0, nj), acc in zip(pair, accs):
                    nc.scalar.activation(
                        o_t[:m, j0 : j0 + nj],
                        acc[:m, :nj],
                        mybir.ActivationFunctionType.Identity,
                        bias=b_t[:m, 0:1],
                    )
            # store in row-chunks, each contiguous in DRAM; SWDGE round-robins
            # one DMA engine per instruction, so 8 chunks engage 8 engines
            n_chunk = 8
            step = (m + n_chunk - 1) // n_chunk
            for r0 in range(0, m, step):
                r1 = min(r0 + step, m)
                nc.gpsimd.dma_start(
                    y_d[t : t + 1, r0 * W : r1 * W], o_t[r0:r1, :]
                )
    nc.compile()
    return nc


def _toeplitz(weight):
    t = np.zeros((128, KW * T_STRIDE), dtype=np.float32)
    idx = np.arange(M_BLK)
    for b in range(KW):
        for a in range(KH):
            t[idx + a, b * T_STRIDE + idx] = weight[a, b]
    return t


def _make_in_maps(x, weight, bias):
    x = np.ascontiguousarray(np.asarray(x, dtype=np.float32))
    weight = np.asarray(weight, dtype=np.float32)
    bias = np.asarray(bias, dtype=np.float32)

    tw = _toeplitz(weight)
    bias_full = np.full((128, 1), float(bias[0]), dtype=np.float32)

    pad_rows = (NCORES - 1) * RPC + IN_ROWS - H  # rows past the image end
    x_pad = np.concatenate(
        [x, np.zeros((pad_rows, W), dtype=np.float32)], axis=0
    )
    in_maps = []
    for c in range(NCORES):
        r0 = c * RPC
        in_maps.append(
            {
                "x": np.ascontiguousarray(x_pad[r0 : r0 + IN_ROWS]),
                "tw": tw,
                "bias": bias_full,
            }
        )
    return in_maps


def _stitch(results):
    y = np.empty((NCORES * RPC, OW), dtype=np.float32)
    for c, r in enumerate(results):
        s = r["y"].reshape(N_BLOCKS, M_BLK, W)
        r0 = c * RPC
        for t in range(N_BLOCKS):
            m = min(M_BLK, RPC - t * M_BLK)
            y[r0 + t * M_BLK : r0 + t * M_BLK + m] = s[t, :m, :OW]
    return y[:OH]


def run_sharded(x, weight, bias, mm_dt=F32R, trace=False, trace_cores=None):
    """Returns (y_full, BassKernelResults)."""
    key = str(mm_dt)
    if key not in _cache:
        _cache[key] = _build(mm_dt)
    nc = _cache[key]
    in_maps = _make_in_maps(x, weight, bias)
    res = bass_utils.run_bass_kernel_spmd(
        nc, in_maps, list(range(NCORES)), trace=trace, trace_cores=trace_cores
    )
    return _stitch(res.results), res


def kernel(x, weight, bias):
    y, _ = run_sharded(x, weight, bias)
    return y


# revision 12
# speedup vs baseline: 1.0335x; 1.0335x over previous
"""2D valid cross-correlation (16x16 kernel, single channel) on 8 TRN2 cores.

Strategy: shard output rows across the 8 cores; each core's input slice
carries a 15-row halo (built host-side from the full image, so no on-device
halo exchange is needed). On each core the conv is computed on the tensor
engine as banded-Toeplitz matmuls contracting over image rows:

    y[i0+m, j0+n] = sum_b sum_k T_b[k, m] * x[i0+k, j0+n+b]

where T_b[k, m] = w[k-m, b] for 0 <= k-m < 16 (built host-side from the
runtime weights). For each output tile of [113 rows x 512 cols], 16
matmuls (one per kernel column b) accumulate into one PSUM bank; the shift
by b is a free-dim offset on the rhs access pattern, so the same SBUF
image tile serves all 16 taps. float32r runs the PE at 1 cycle/row
(~3.2x faster than fp32 matmul) at ~1e-3 scale-relative accuracy.

Output rows of a block are evicted into one wide SBUF tile and stored as a
single contiguous DRAM transfer per block (strided small stores serialize
onto one DMA engine at ~22 GB/s; contiguous transfers fan out across SDMA
engines). The scratch [block, 113*4096] layout is re-stitched on the host.
"""

from contextlib import ExitStack

import numpy as np

import concourse.tile as tile
from concourse import bacc, mybir
from concourse import bass_utils

H = W = 4096
KH = KW = 16
OH = OW = H - KH + 1          # 4081
NCORES = 8
RPC = 511                     # output rows per core (8*511 = 4088 >= 4081)
M_BLK = 113                   # output rows per lhsT block (128 - (KH-1))
T_STRIDE = 114                # per-tap stride in the Toeplitz tile (8B-aligned)
IN_ROWS = RPC + KH - 1        # 526 input rows per core (15-row halo)
N_TILE = 512                  # output cols per PSUM tile (one fp32 bank)
N_BLOCKS = (RPC + M_BLK - 1) // M_BLK  # 5 (last block: m=59)
# fp32r matmuls need even free counts; last tile is 498 wide with a
# 1-column overlap (col 3583 written twice with the identical value).
W_TILES = [(j, 512) for j in range(0, 3584, 512)] + [(3583, 498)]

F32 = mybir.dt.float32
F32R = mybir.dt.float32r

_cache = {}


def _build(mm_dt):
    nc = bacc.Bacc("TRN2", target_bir_lowering=False, debug=False)
    x_d = nc.dram_tensor("x", [IN_ROWS, W], mm_dt, kind="ExternalInput")
    t_d = nc.dram_tensor("tw", [128, KW * T_STRIDE], mm_dt, kind="ExternalInput")
    b_d = nc.dram_tensor("bias", [128, 1], F32, kind="ExternalInput")
    y_d = nc.dram_tensor("y", [N_BLOCKS, M_BLK * W], F32, kind="ExternalOutput")

    with tile.TileContext(nc) as tc, ExitStack() as ctx:
        const_pool = ctx.enter_context(tc.tile_pool(name="const", bufs=1))
        x_pool = ctx.enter_context(tc.tile_pool(name="xblk", bufs=2))
        ev_pool = ctx.enter_context(tc.tile_pool(name="evict", bufs=3))
        ps_pool = ctx.enter_context(tc.tile_pool(name="acc", bufs=8, space="PSUM"))

        # T/bias on the scalar HWDGE ring so they transfer in parallel with
        # block 0's image rows on the sync ring
        t_t = const_pool.tile([128, KW * T_STRIDE], mm_dt)
        nc.scalar.dma_start(t_t[:], t_d[:])
        b_t = const_pool.tile([128, 1], F32)
        nc.scalar.dma_start(b_t[:], b_d[:])

        for t in range(N_BLOCKS):
            m = min(M_BLK, RPC - t * M_BLK)
            k = m + KH - 1
            x_t = x_pool.tile([128, W], mm_dt, tag="xblk")
            nc.sync.dma_start(x_t[:k, :], x_d[t * M_BLK : t * M_BLK + k, :])
            o_t = ev_pool.tile([M_BLK, W], F32, tag="out")
            for j0, nj in W_TILES:
                acc = ps_pool.tile([M_BLK, N_TILE], F32, tag="acc")
                for b in range(KW):
                    nc.tensor.matmul(
                        acc[:m, :nj],
                        t_t[:k, b * T_STRIDE : b * T_STRIDE + m],
                        x_t[:k, j0 + b : j0 + b + nj],
                        start=(b == 0),
                        stop=(b == KW - 1),
                    )
                nc.scalar.activation(
                    o_t[:m, j0 : j0 + nj],
                    acc[:m, :nj],
                    mybir.ActivationFunctionType.Identity,
                    bias=b_t[:m, 0:1],
                )
            # store in row-chunks, each contiguous in DRAM; SWDGE round-robins
            # one DMA engine per instruction, so 8 chunks engage 8 engines
            n_chunk = 8
            step = (m + n_chunk - 1) // n_chunk
            for r0 in range(0, m, step):
                r1 = min(r0 + step, m)
                nc.gpsimd.dma_start(
                    y_d[t : t + 1, r0 * W : r1 * W], o_t[r0:r1, :]
                )
    nc.compile()
    return nc


def _toeplitz(weight):
    t = np.zeros((128, KW * T_STRIDE), dtype=np.float32)
    idx = np.arange(M_BLK)
    for b in range(KW):
        for a in range(KH):
            t[idx + a, b * T_STRIDE + idx] = weight[a, b]
    return t


def _make_in_maps(x, weight, bias):
    x = np.ascontiguousarray(np.asarray(x, dtype=np.float32))
    weight = np.asarray(weight, dtype=np.float32)
    bias = np.asarray(bias, dtype=np.float32)

    tw = _toeplitz(weight)
    bias_full = np.full((128, 1), float(bias[0]), dtype=np.float32)

    pad_rows = (NCORES - 1) * RPC + IN_ROWS - H  # rows past the image end
    x_pad = np.concatenate(
        [x, np.zeros((pad_rows, W), dtype=np.float32)], axis=0
    )
    in_maps = []
    for c in range(NCORES):
        r0 = c * RPC
        in_maps.append(
            {
                "x": np.ascontiguousarray(x_pad[r0 : r0 + IN_ROWS]),
                "tw": tw,
                "bias": bias_full,
            }
        )
    return in_maps


def _stitch(results):
    y = np.empty((NCORES * RPC, OW), dtype=np.float32)
    for c, r in enumerate(results):
        s = r["y"].reshape(N_BLOCKS, M_BLK, W)
        r0 = c * RPC
        for t in range(N_BLOCKS):
            m = min(M_BLK, RPC - t * M_BLK)
            y[r0 + t * M_BLK : r0 + t * M_BLK + m] = s[t, :m, :OW]
    return y[:OH]


def run_sharded(x, weight, bias, mm_dt=F32R, trace=False, trace_cores=None):
    """Returns (y_full, BassKernelResults)."""
    key = str(mm_dt)
    if key not in _cache:
        _cache[key] = _build(mm_dt)
    nc = _cache[key]
    in_maps = _make_in_maps(x, weight, bias)
    res = bass_utils.run_bass_kernel_spmd(
        nc, in_maps, list(range(NCORES)), trace=trace, trace_cores=trace_cores
    )
    return _stitch(res.results), res


def kernel(x, weight, bias):
    y, _ = run_sharded(x, weight, bias)
    return y


# revision 14
# speedup vs baseline: 1.1585x; 1.1209x over previous
"""2D valid cross-correlation (16x16 kernel, single channel) on 8 TRN2 cores.

Strategy: shard output rows across the 8 cores; each core's input slice
carries a 15-row halo (built host-side from the full image, so no on-device
halo exchange is needed). On each core the conv is computed on the tensor
engine as banded-Toeplitz matmuls contracting over image rows:

    y[i0+m, j0+n] = sum_b sum_k T_b[k, m] * x[i0+k, j0+n+b]

where T_b[k, m] = w[k-m, b] for 0 <= k-m < 16 (built host-side from the
runtime weights). For each output tile of [113 rows x 512 cols], 16
matmuls (one per kernel column b) accumulate into one PSUM bank; the shift
by b is a free-dim offset on the rhs access pattern, so the same SBUF
image tile serves all 16 taps. float32r runs the PE at 1 cycle/row
(~3.2x faster than fp32 matmul) at ~1e-3 scale-relative accuracy.

Output rows of a block are evicted into one wide SBUF tile and stored as a
single contiguous DRAM transfer per block (strided small stores serialize
onto one DMA engine at ~22 GB/s; contiguous transfers fan out across SDMA
engines). The scratch [block, 113*4096] layout is re-stitched on the host.
"""

from contextlib import ExitStack

import numpy as np

import concourse.tile as tile
from concourse import bacc, mybir
from concourse import bass_utils

H = W = 4096
KH = KW = 16
OH = OW = H - KH + 1          # 4081
NCORES = 8
RPC = 511                     # output rows per core (8*511 = 4088 >= 4081)
M_BLK = 113                   # output rows per lhsT block (128 - (KH-1))
T_STRIDE = 114                # per-tap stride in the Toeplitz tile (8B-aligned)
N_TILE = 512                  # output cols per PSUM tile (one fp32 bank)
N_BLOCKS = (RPC + M_BLK - 1) // M_BLK  # 5 (last block: 59 valid rows)
# All blocks run full K=128/M=113 shapes: partial fp32r matmuls (K=74/M=59)
# were measured at half speed (427ns vs 234ns at N=512). The input is
# zero-padded host-side; garbage output rows are dropped in the stitch.
IN_ROWS = N_BLOCKS * M_BLK + KH - 1  # 580 input rows per core
# fp32r matmuls need even free counts; last tile is 498 wide with a
# 1-column overlap (col 3583 written twice with the identical value).
W_TILES = [(j, 512) for j in range(0, 3584, 512)] + [(3583, 498)]

F32 = mybir.dt.float32
F32R = mybir.dt.float32r

_cache = {}


def _build(mm_dt):
    nc = bacc.Bacc("TRN2", target_bir_lowering=False, debug=False)
    x_d = nc.dram_tensor("x", [IN_ROWS, W], mm_dt, kind="ExternalInput")
    t_d = nc.dram_tensor("tw", [128, KW * T_STRIDE], mm_dt, kind="ExternalInput")
    b_d = nc.dram_tensor("bias", [128, 1], F32, kind="ExternalInput")
    y_d = nc.dram_tensor("y", [N_BLOCKS, M_BLK * W], F32, kind="ExternalOutput")

    with tile.TileContext(nc) as tc, ExitStack() as ctx:
        const_pool = ctx.enter_context(tc.tile_pool(name="const", bufs=1))
        x_pool = ctx.enter_context(tc.tile_pool(name="xblk", bufs=2))
        ev_pool = ctx.enter_context(tc.tile_pool(name="evict", bufs=3))
        ps_pool = ctx.enter_context(tc.tile_pool(name="acc", bufs=8, space="PSUM"))

        # T/bias on the scalar HWDGE ring so they transfer in parallel with
        # block 0's image rows on the sync ring
        t_t = const_pool.tile([128, KW * T_STRIDE], mm_dt)
        nc.scalar.dma_start(t_t[:], t_d[:])
        b_t = const_pool.tile([128, 1], F32)
        nc.scalar.dma_start(b_t[:], b_d[:])

        for t in range(N_BLOCKS):
            m = M_BLK
            k = m + KH - 1  # 128
            x_t = x_pool.tile([128, W], mm_dt, tag="xblk")
            nc.sync.dma_start(x_t[:k, :], x_d[t * M_BLK : t * M_BLK + k, :])
            o_t = ev_pool.tile([M_BLK, W], F32, tag="out")
            for j0, nj in W_TILES:
                acc = ps_pool.tile([M_BLK, N_TILE], F32, tag="acc")
                for b in range(KW):
                    nc.tensor.matmul(
                        acc[:m, :nj],
                        t_t[:k, b * T_STRIDE : b * T_STRIDE + m],
                        x_t[:k, j0 + b : j0 + b + nj],
                        start=(b == 0),
                        stop=(b == KW - 1),
                    )
                nc.scalar.activation(
                    o_t[:m, j0 : j0 + nj],
                    acc[:m, :nj],
                    mybir.ActivationFunctionType.Identity,
                    bias=b_t[:m, 0:1],
                )
            # store in row-chunks, each contiguous in DRAM; SWDGE round-robins
            # one DMA engine per instruction, so 8 chunks engage 8 engines
            n_chunk = 8
            step = (m + n_chunk - 1) // n_chunk
            for r0 in range(0, m, step):
                r1 = min(r0 + step, m)
                nc.gpsimd.dma_start(
                    y_d[t : t + 1, r0 * W : r1 * W], o_t[r0:r1, :]
                )
    nc.compile()
    return nc


def _toeplitz(weight):
    t = np.zeros((128, KW * T_STRIDE), dtype=np.float32)
    idx = np.arange(M_BLK)
    for b in range(KW):
        for a in range(KH):
            t[idx + a, b * T_STRIDE + idx] = weight[a, b]
    return t


def _make_in_maps(x, weight, bias):
    x = np.ascontiguousarray(np.asarray(x, dtype=np.float32))
    weight = np.asarray(weight, dtype=np.float32)
    bias = np.asarray(bias, dtype=np.float32)

    tw = _toeplitz(weight)
    bias_full = np.full((128, 1), float(bias[0]), dtype=np.float32)

    pad_rows = (NCORES - 1) * RPC + IN_ROWS - H  # rows past the image end
    x_pad = np.concatenate(
        [x, np.zeros((pad_rows, W), dtype=np.float32)], axis=0
    )
    in_maps = []
    for c in range(NCORES):
        r0 = c * RPC
        in_maps.append(
            {
                "x": np.ascontiguousarray(x_pad[r0 : r0 + IN_ROWS]),
                "tw": tw,
                "bias": bias_full,
            }
        )
    return in_maps


def _stitch(results):
    y = np.empty((NCORES * RPC, OW), dtype=np.float32)
    for c, r in enumerate(results):
        s = r["y"].reshape(N_BLOCKS, M_BLK, W)
        r0 = c * RPC
        for t in range(N_BLOCKS):
            m = min(M_BLK, RPC - t * M_BLK)
            y[r0 + t * M_BLK : r0 + t * M_BLK + m] = s[t, :m, :OW]
    return y[:OH]


def run_sharded(x, weight, bias, mm_dt=F32R, trace=False, trace_cores=None):
    """Returns (y_full, BassKernelResults)."""
    key = str(mm_dt)
    if key not in _cache:
        _cache[key] = _build(mm_dt)
    nc = _cache[key]
    in_maps = _make_in_maps(x, weight, bias)
    res = bass_utils.run_bass_kernel_spmd(
        nc, in_maps, list(range(NCORES)), trace=trace, trace_cores=trace_cores
    )
    return _stitch(res.results), res


def kernel(x, weight, bias):
    y, _ = run_sharded(x, weight, bias)
    return y
